# revision 3
# baseline (speedup 1.0000x reference)
"""Expert-choice MoE kernel for 8 Trainium2 NeuronCores (expert-parallel).

Decomposition (core e handles expert e):
  - router logits x . emb_e computed in fp32 on PE; top-8 token indices per
    batch row via DVE max8/max_index; token gather via indirect DMA.
  - all activations/weights in fp16 (same bytes/speed as bf16, 8x less
    rounding noise), except: router fp32, w1 fully in fp8-e3m4 and the
    first 2/3 of w2's columns in fp8-e3m4 (scale 96, folded into x2h and
    the `we` combine weights respectively).
  - weights pre-packed host-side into partition-major [128, chunks, cols]
    layouts and streamed in multi-chunk groups (one DMA + a long matmul
    burst per group) so the PE stays busy long enough to leave the mid
    p-state and the DMA queue runs near the 360 GB/s cap.
  - sum_weights GEMM1 column-sharded (each core owns 1536 columns of
    sw_w1); the (8,64) partial logits are AllReduced, softmaxed locally.
  - expert GEMM2 emitted in 3 column chunks of 1024; each chunk's
    we-scaled er contribution is AllReduced in fp16 so the head GEMM can
    start on chunk j while chunk j+1 is still in flight.
  - classification head sharded: GEMM1 column-shard (384 cols of ch_w1),
    GEMM2 contraction-shard (384 rows of ch_w2); per-core (64,1000)
    partials are summed on the host (+ ch_b2).
"""

import numpy as np
import ml_dtypes

import concourse.bass as bass
from concourse import bacc
import concourse.mybir as mybir
import concourse.tile as tile
from concourse.bass import ts, ds
from concourse.bass_utils import run_bass_kernel_spmd
from concourse.masks import make_identity

B, N, D, E, K, C = 64, 32, 384, 8, 8, 1000
KD, ND = K * D, N * D          # 3072, 12288
P = 128
NTOK = B * N                   # 2048
SWC = ND // E                  # 1536 sum-weights columns per core
CH1C = KD // E                 # 384 head-GEMM1 columns per core
KCE = KD // P                  # 24 k-chunks, expert GEMMs
KCS = ND // P                  # 96 k-chunks, sum-weights GEMM1
KCH = SWC // P                 # 12 k-chunks, z GEMM
NCORES = 8

SWG = 8                        # sw1 k-chunks per DMA group (12 groups)
W1G = 4                        # w1 k-chunks per group (6 groups)
W2CW = 1024                    # w2 column-chunk width (3 chunks)
NCHUNK = KD // W2CW            # 3
W2G8 = 12                      # w2-fp8 k-chunks per group (2 groups/chunk)
W2G16 = 6                      # w2-fp16 k-chunks per group (4 groups/chunk)
NF8 = 2                        # first NF8 of the 3 w2 column chunks are fp8
S1 = 96.0                      # w1 fp8 scale (folded into x2h)
S2 = 96.0                      # w2 fp8 scale (folded into we)

F32 = mybir.dt.float32
F16 = mybir.dt.float16
F8E3 = mybir.dt.float8e3
U32 = mybir.dt.uint32
GELU = mybir.ActivationFunctionType.Gelu
EXP = mybir.ActivationFunctionType.Exp
X_AX = mybir.AxisListType.X
ADD = mybir.AluOpType.add
f16 = np.float16
e3m4 = ml_dtypes.float8_e3m4


def _build(include_bias: bool) -> bass.Bass:
    nc = bacc.Bacc("TRN2", num_devices=NCORES)

    xt = nc.dram_tensor("xt", [P, 3, NTOK + 1], F32, kind="ExternalInput")
    x2h = nc.dram_tensor("x2h", [NTOK, D], F16, kind="ExternalInput")
    xft = nc.dram_tensor("xft", [P, KCS, B], F16, kind="ExternalInput")
    sw1p = nc.dram_tensor("sw1p", [P, KCS, SWC], F16, kind="ExternalInput")
    w1p = nc.dram_tensor("w1p", [P, KCE, KD], F8E3, kind="ExternalInput")
    w2p8 = nc.dram_tensor("w2p8", [NF8, P, KCE, W2CW], F8E3, kind="ExternalInput")
    w2p16 = nc.dram_tensor(
        "w2p16", [NCHUNK - NF8, P, KCE, W2CW], F16, kind="ExternalInput")
    sw2p = nc.dram_tensor("sw2p", [P, KCH, E], F16, kind="ExternalInput")
    ch1p = nc.dram_tensor("ch1p", [P, KCE, CH1C], F16, kind="ExternalInput")
    ch2p = nc.dram_tensor("ch2p", [P, 3, C], F16, kind="ExternalInput")
    oh = nc.dram_tensor("oh", [B, E], F32, kind="ExternalInput")
    if include_bias:
        b1d = nc.dram_tensor("b1d", [1, KD], F32, kind="ExternalInput")
        b2d = nc.dram_tensor("b2d", [1, KD], F32, kind="ExternalInput")
        swb1d = nc.dram_tensor("swb1d", [1, SWC], F32, kind="ExternalInput")
        swb2d = nc.dram_tensor("swb2d", [1, E], F32, kind="ExternalInput")
        chb1d = nc.dram_tensor("chb1d", [1, CH1C], F32, kind="ExternalInput")
    outp = nc.dram_tensor("outp", [B, C], F32, kind="ExternalOutput")

    with tile.TileContext(nc) as tc:
        with (
            tc.tile_pool(name="consts", bufs=1) as consts,
            tc.tile_pool(name="acts", bufs=1) as acts,
            tc.tile_pool(name="wpool", bufs=3) as wpool,
            tc.tile_pool(name="erpool", bufs=2) as erpool,
            tc.tile_pool(name="ps_mm", bufs=6, space="PSUM") as ps_mm,
            tc.tile_pool(name="ps_tr", bufs=2, space="PSUM") as ps_tr,
            tc.tile_pool(name="dram", bufs=1, space="DRAM") as dram,
        ):
            # ---- constants / persistent activations ----
            ident = consts.tile([P, P], F16)
            make_identity(nc, ident[:])
            xft_sb = consts.tile([P, KCS, B], F16)
            nc.sync.dma_start(xft_sb[:], xft[:])
            sw2_sb = consts.tile([P, KCH, E], F16)
            nc.sync.dma_start(sw2_sb[:], sw2p[:])
            oh_sb = consts.tile([B, E], F32)
            nc.sync.dma_start(oh_sb[:], oh[:])
            pwarm = ps_tr.tile([P, B], F16, name="pwarm", tag="pt")
            nc.tensor.transpose(pwarm[:32, :32], ident[:32, :32], ident[:32, :32])
            if include_bias:
                b1_sb = consts.tile([B, KD], F32)
                nc.scalar.dma_start(b1_sb[:], b1d[0:1, :].to_broadcast([B, KD]))
                b2_sb = consts.tile([B, KD], F32)
                nc.scalar.dma_start(b2_sb[:], b2d[0:1, :].to_broadcast([B, KD]))
                swb1_sb = consts.tile([B, SWC], F32)
                nc.scalar.dma_start(swb1_sb[:], swb1d[0:1, :].to_broadcast([B, SWC]))
                swb2_sb = consts.tile([B, E], F32)
                nc.scalar.dma_start(swb2_sb[:], swb2d[0:1, :].to_broadcast([B, E]))
                chb1_sb = consts.tile([B, CH1C], F32)
                nc.scalar.dma_start(chb1_sb[:], chb1d[0:1, :].to_broadcast([B, CH1C]))

            # ---- phase 1: sum-weights GEMM1, 12 groups of 8 k-chunks ----
            pms = [ps_mm.tile([B, 512], F32, name=f"pms{n}", tag="pm")
                   for n in range(3)]

            def sw_group(g):
                wt = wpool.tile([P, SWG, SWC], F16, name="wt", tag="wt")
                nc.sync.dma_start(wt[:], sw1p[:, ts(g, SWG), :])
                for c8 in range(SWG):
                    c = g * SWG + c8
                    for n in range(3):
                        nc.tensor.matmul(
                            pms[n][:], xft_sb[:, c, :], wt[:, c8, ts(n, 512)],
                            start=(c == 0), stop=(c == KCS - 1),
                        )

            sw_group(0)
            xt_sb = acts.tile([P, 3, NTOK + 1], F32)
            nc.sync.dma_start(xt_sb[:], xt[:])
            sw_group(1)

            # ---- router: logits = x @ emb_e, fp32 (emb packed as col 2048) ----
            lg_flat = acts.tile([1, NTOK], F32)
            for nt in range(4):
                pr = ps_mm.tile([B, 512], F32, name="pr", tag="pm")
                for c in range(3):
                    nc.tensor.matmul(
                        pr[:1, :], xt_sb[:, c, NTOK : NTOK + 1],
                        xt_sb[:, c, ts(nt, 512)],
                        start=(c == 0), stop=(c == 2),
                    )
                nc.vector.tensor_copy(lg_flat[:, ts(nt, 512)], pr[:1, :])
            lg_dram = dram.tile([1, NTOK], F32)
            nc.scalar.dma_start(lg_dram[:], lg_flat[:])
            lg_bn = acts.tile([B, N], F32)
            nc.scalar.dma_start(lg_bn[:], lg_dram[:].rearrange("x (b n) -> (x b) n", b=B))

            # ---- top-8 tokens per row + gather (overlaps sw streaming) ----
            vals8 = acts.tile([B, 8], F32)
            idx8 = acts.tile([B, 8], U32)
            nc.vector.max(out=vals8[:], in_=lg_bn[:])
            nc.vector.max_index(out=idx8[:], in_max=vals8[:], in_values=lg_bn[:])
            base = acts.tile([B, 1], U32)
            nc.gpsimd.iota(base[:], pattern=[[0, 1]], base=0, channel_multiplier=N)
            off = acts.tile([B, 8], U32)
            nc.vector.tensor_tensor(
                out=off[:], in0=idx8[:], in1=base[:].to_broadcast([B, 8]), op=ADD
            )
            sel = acts.tile([B, K, D], F16)
            for k in range(K):
                nc.gpsimd.indirect_dma_start(
                    out=sel[:, k, :], out_offset=None,
                    in_=x2h[:],
                    in_offset=bass.IndirectOffsetOnAxis(ap=off[:, k : k + 1], axis=0),
                )
            sel_flat = sel[:].rearrange("b k d -> b (k d)")

            # ---- rest of the sw stream; selT transposes sprinkled in ----
            selT = acts.tile([P, KCE, B], F16)
            for g in range(2, KCS // SWG):
                sw_group(g)
                if g >= 6:
                    for cc in range(4):
                        c = (g - 6) * 4 + cc
                        pt = ps_tr.tile([P, B], F16, name="pt", tag="pt")
                        nc.tensor.transpose(pt[:], sel_flat[:, ts(c, P)], ident[:B, :B])
                        nc.vector.tensor_copy(selT[:, c, :], pt[:])
            ch2_sb = consts.tile([P, 3, C], F16)
            nc.sync.dma_start(ch2_sb[:], ch2p[:])

            # ---- h1 = gelu(sw psums); z partial; z AllReduce ----
            h1 = acts.tile([B, SWC], F16)
            for n in range(3):
                if include_bias:
                    nc.vector.tensor_add(pms[n][:], pms[n][:], swb1_sb[:, ts(n, 512)])
                nc.scalar.activation(h1[:, ts(n, 512)], pms[n][:], GELU)
            h1T = acts.tile([P, KCH, B], F16)
            for c in range(KCH):
                pt = ps_tr.tile([P, B], F16, name="pt", tag="pt")
                nc.tensor.transpose(pt[:], h1[:, ts(c, P)], ident[:B, :B])
                nc.vector.tensor_copy(h1T[:, c, :], pt[:])
            pz = ps_mm.tile([E, B], F32, name="pz", tag="pm")
            for c in range(KCH):
                nc.tensor.matmul(
                    pz[:], sw2_sb[:, c, :], h1T[:, c, :],
                    start=(c == 0), stop=(c == KCH - 1),
                )
            zT_sb = acts.tile([E, B], F32)
            nc.vector.tensor_copy(zT_sb[:], pz[:])
            zin = dram.tile([E, B], F32)
            zout = dram.tile([E, B], F32)
            nc.gpsimd.dma_start(zin[:], zT_sb[:])
            nc.gpsimd.collective_compute(
                "AllReduce", ADD, replica_groups=[list(range(NCORES))],
                ins=[zin[:].opt()], outs=[zout[:].opt()],
            )

            # ---- phase 2: expert GEMM1, fp8 w1 stream (6 groups of 4) ----
            h = acts.tile([B, KD], F16)
            pme = [ps_mm.tile([B, 512], F32, name=f"pme{n}", tag="pm")
                   for n in range(6)]
            for g in range(KCE // W1G):
                wt8 = wpool.tile([P, W1G, KD], F8E3, name="wt8", tag="wt")
                nc.sync.dma_start(wt8[:], w1p[:, ts(g, W1G), :])
                for c4 in range(W1G):
                    c = g * W1G + c4
                    for n in range(6):
                        nc.tensor.matmul(
                            pme[n][:], selT[:, c, :], wt8[:, c4, ts(n, 512)],
                            start=(c == 0), stop=(c == KCE - 1),
                        )
            last_gelu = None
            for n in range(6):
                if include_bias:
                    nc.vector.tensor_add(pme[n][:], pme[n][:], b1_sb[:, ts(n, 512)])
                last_gelu = nc.scalar.activation(h[:, ts(n, 512)], pme[n][:], GELU)
            hT = acts.tile([P, KCE, B], F16)
            last_htc = None
            for c in range(KCE):
                pt = ps_tr.tile([P, B], F16, name="pt", tag="pt")
                nc.tensor.transpose(pt[:], h[:, ts(c, P)], ident[:B, :B])
                last_htc = nc.vector.tensor_copy(hT[:, c, :], pt[:])

            # softmax over experts -> w; we = w[:, e] via one-hot. Ordering
            # deps keep the z-AllReduce consumers BEHIND the expert-GEMM work
            # on the ACT/DVE queues (the scheduler would otherwise hoist them
            # and park those queues on the collective).
            zb = acts.tile([B, E], F32)
            nc.gpsimd.dma_start(zb[:], zout[:].rearrange("e b -> b e"))
            if include_bias:
                nc.vector.tensor_add(zb[:], zb[:], swb2_sb[:])
            mx = acts.tile([B, 1], F32)
            mx_i = nc.vector.reduce_max(mx[:], zb[:], axis=X_AX)
            tile.add_dep_helper(mx_i.ins, last_htc.ins, sync=False,
                                reason="softmax after hT copies on DVE")
            nmx = acts.tile([B, 1], F32)
            nc.vector.tensor_scalar_mul(nmx[:], mx[:], -1.0)
            exps = acts.tile([B, E], F32)
            exp_i = nc.scalar.activation(exps[:], zb[:], EXP, bias=nmx[:])
            tile.add_dep_helper(exp_i.ins, last_gelu.ins, sync=False,
                                reason="Exp after expert gelus on ACT")
            sm = acts.tile([B, 1], F32)
            nc.vector.reduce_sum(sm[:], exps[:], axis=X_AX)
            rs = acts.tile([B, 1], F32)
            nc.vector.reciprocal(rs[:], sm[:])
            wv = acts.tile([B, E], F32)
            nc.vector.tensor_scalar_mul(wv[:], exps[:], rs[:])
            t8 = acts.tile([B, E], F32)
            nc.vector.tensor_mul(out=t8[:], in0=wv[:], in1=oh_sb[:])
            we = acts.tile([B, 1], F32)
            nc.vector.reduce_sum(we[:], t8[:], axis=X_AX)
            we8 = acts.tile([B, 1], F32)
            nc.vector.tensor_scalar_mul(we8[:], we[:], 1.0 / S2)

            # ---- phase 3: expert GEMM2 in 3 col chunks, pipelined ARs ----
            ch1_sb = acts.tile([P, KCE, CH1C], F16)
            cins = [dram.tile([B, W2CW], F16, name=f"cin{j}") for j in range(NCHUNK)]
            wsouts = [dram.tile([B, W2CW], F16, name=f"wsout{j}")
                      for j in range(NCHUNK)]
            last_ermm = None
            for j in range(NCHUNK):
                pm2 = [ps_mm.tile([B, 512], F32, name=f"pm2{j}{n}", tag="pm")
                       for n in range(2)]
                if j < NF8:
                    ng, gsz, wdt = KCE // W2G8, W2G8, F8E3
                else:
                    ng, gsz, wdt = KCE // W2G16, W2G16, F16
                for g in range(ng):
                    wt2 = wpool.tile([P, gsz, W2CW], wdt, name="wt2", tag="wt")
                    src = w2p8 if j < NF8 else w2p16
                    nc.sync.dma_start(wt2[:], src[j - (0 if j < NF8 else NF8), :, ts(g, gsz), :])
                    for cg in range(gsz):
                        c = g * gsz + cg
                        for n in range(2):
                            last_ermm = nc.tensor.matmul(
                                pm2[n][:], hT[:, c, :], wt2[:, cg, ts(n, 512)],
                                start=(c == 0), stop=(c == KCE - 1),
                            )
                # stream the head GEMM1 weights while the ARs fly
                nc.sync.dma_start(ch1_sb[:, ts(j, SWG), :], ch1p[:, ts(j, SWG), :])
                wej = we8 if j < NF8 else we
                er_j = erpool.tile([B, W2CW], F16, name="er", tag="er")
                for n in range(2):
                    col = n * 512
                    if include_bias:
                        stage = acts.tile([B, 512], F32, name=f"stage{j}{n}")
                        nc.vector.tensor_scalar_mul(stage[:], pm2[n][:], wej[:])
                        tmpb = acts.tile([B, 512], F32, name=f"tmpb{j}{n}")
                        nc.vector.tensor_scalar_mul(
                            tmpb[:], b2_sb[:, ds(j * W2CW + col, 512)], we[:])
                        nc.vector.tensor_add(stage[:], stage[:], tmpb[:])
                        nc.vector.tensor_copy(er_j[:, ds(col, 512)], stage[:])
                    else:
                        nc.vector.tensor_scalar_mul(
                            er_j[:, ds(col, 512)], pm2[n][:], wej[:])
                nc.scalar.dma_start(cins[j][:], er_j[:])
                nc.gpsimd.collective_compute(
                    "AllReduce", ADD, replica_groups=[list(range(NCORES))],
                    ins=[cins[j][:].opt()], outs=[wsouts[j][:].opt()],
                )

            # ---- phase 4: head GEMM1 per ws chunk as its AR lands ----
            ws = acts.tile([B, KD], F16)
            wsT = acts.tile([P, KCE, B], F16)
            hh32 = acts.tile([B, CH1C], F32)
            for j in range(NCHUNK):
                nc.scalar.dma_start(ws[:, ts(j, W2CW)], wsouts[j][:])
                pmh = ps_mm.tile([B, CH1C], F32, name=f"pmh{j}", tag="pm")
                for cc in range(SWG):
                    c = j * SWG + cc
                    pt = ps_tr.tile([P, B], F16, name="pt", tag="pt")
                    tr_i = nc.tensor.transpose(
                        pt[:], ws[:, ts(c, P)], ident[:B, :B])
                    if j == 0 and cc == 0 and last_ermm is not None:
                        tile.add_dep_helper(tr_i.ins, last_ermm.ins, sync=False,
                                            reason="head transposes after er matmuls on PE")
                    nc.vector.tensor_copy(wsT[:, c, :], pt[:])
                    nc.tensor.matmul(
                        pmh[:], wsT[:, c, :], ch1_sb[:, c, :],
                        start=(cc == 0), stop=(cc == SWG - 1),
                    )
                if j == 0:
                    nc.vector.tensor_copy(hh32[:], pmh[:])
                else:
                    nc.vector.tensor_add(hh32[:], hh32[:], pmh[:])
            if include_bias:
                nc.vector.tensor_add(hh32[:], hh32[:], chb1_sb[:])
            hh = acts.tile([B, CH1C], F16)
            nc.scalar.activation(hh[:], hh32[:], GELU)

            # ---- head GEMM2 (contraction shard): out_part = hh @ ch2_e ----
            hhT = acts.tile([P, 3, B], F16)
            for c in range(3):
                pt = ps_tr.tile([P, B], F16, name="pt", tag="pt")
                nc.tensor.transpose(pt[:], hh[:, ts(c, P)], ident[:B, :B])
                nc.vector.tensor_copy(hhT[:, c, :], pt[:])
            outsb = acts.tile([B, C], F32)
            for nn in range(2):
                pmo = ps_mm.tile([B, 512], F32, name="pmo", tag="pm")
                for c in range(3):
                    nc.tensor.matmul(
                        pmo[:, :500], hhT[:, c, :], ch2_sb[:, c, ds(nn * 500, 500)],
                        start=(c == 0), stop=(c == 2),
                    )
                nc.vector.tensor_copy(outsb[:, ds(nn * 500, 500)], pmo[:, :500])
            nc.sync.dma_start(outp[:], outsb[:])

    nc.finalize()
    return nc


_NC_CACHE: dict = {}


def _get_nc(include_bias: bool) -> bass.Bass:
    if include_bias not in _NC_CACHE:
        _NC_CACHE[include_bias] = _build(include_bias)
    return _NC_CACHE[include_bias]


def _pmajor(a, nchunk, dt):
    """[nchunk*128, cols] -> [128, nchunk, cols] partition-major, cast."""
    cols = a.shape[1]
    return np.ascontiguousarray(
        a.reshape(nchunk, P, cols).transpose(1, 0, 2)).astype(dt)


def _pack_inputs(inputs: dict, include_bias: bool) -> list[dict]:
    fp32 = np.float32
    x = np.ascontiguousarray(np.asarray(inputs["x"], dtype=fp32))  # (64,32,384)
    expert_emb = np.asarray(inputs["expert_emb"], dtype=fp32)      # (8,384)
    w1 = np.asarray(inputs["w1"], fp32)                            # (8,3072,3072)
    w2 = np.asarray(inputs["w2"], fp32)
    sw_w1 = np.asarray(inputs["sw_w1"], fp32)                      # (12288,12288)
    sw_w2 = np.asarray(inputs["sw_w2"], fp32)                      # (12288,8)
    ch_w1 = np.asarray(inputs["ch_w1"], fp32)                      # (3072,3072)
    ch_w2 = np.asarray(inputs["ch_w2"], fp32)                      # (3072,1000)

    x2 = x.reshape(NTOK, D)
    xt_base = x2.T.reshape(3, P, NTOK).transpose(1, 0, 2)          # (128,3,2048)
    x2h = (x2 / S1).astype(f16)                                    # (2048,384)
    xf = x.reshape(B, ND)
    xft_p = _pmajor(np.ascontiguousarray(xf.T), KCS, f16)          # (128,96,64)

    in_maps = []
    for e in range(NCORES):
        emb_p = expert_emb[e].reshape(3, P).T                      # (128,3)
        xt_p = np.ascontiguousarray(
            np.concatenate([xt_base, emb_p[:, :, None]], axis=2), dtype=fp32)
        sw1_e = np.ascontiguousarray(sw_w1[:, e * SWC:(e + 1) * SWC])
        sw1_p = _pmajor(sw1_e, KCS, f16)                           # (128,96,1536)
        w1_p = _pmajor(w1[e] * S1, KCE, e3m4)                      # (128,24,3072)
        w2p8 = np.stack([
            _pmajor(w2[e][:, j * W2CW:(j + 1) * W2CW] * S2, KCE, e3m4)
            for j in range(NF8)])
        w2p16 = np.stack([
            _pmajor(w2[e][:, j * W2CW:(j + 1) * W2CW], KCE, f16)
            for j in range(NF8, NCHUNK)])
        sw2_e = np.ascontiguousarray(sw_w2[e * SWC:(e + 1) * SWC, :])
        sw2_p = _pmajor(sw2_e, KCH, f16)                           # (128,12,8)
        ch1_p = _pmajor(
            np.ascontiguousarray(ch_w1[:, e * CH1C:(e + 1) * CH1C]), KCE, f16)
        ch2_p = _pmajor(
            np.ascontiguousarray(ch_w2[e * CH1C:(e + 1) * CH1C, :]), 3, f16)
        oh_p = np.zeros((B, E), dtype=fp32)
        oh_p[:, e] = 1.0
        m = {
            "xt": xt_p, "x2h": x2h, "xft": xft_p, "sw1p": sw1_p,
            "w1p": w1_p, "w2p8": w2p8, "w2p16": w2p16,
            "sw2p": sw2_p, "ch1p": ch1_p, "ch2p": ch2_p, "oh": oh_p,
        }
        if include_bias:
            m["b1d"] = np.asarray(inputs["b1"][e], fp32).reshape(1, KD)
            m["b2d"] = np.asarray(inputs["b2"][e], fp32).reshape(1, KD)
            m["swb1d"] = np.asarray(
                inputs["sw_b1"], fp32).reshape(1, ND)[:, e * SWC:(e + 1) * SWC]
            m["swb2d"] = np.asarray(inputs["sw_b2"], fp32).reshape(1, E)
            m["chb1d"] = np.asarray(
                inputs["ch_b1"], fp32).reshape(1, KD)[:, e * CH1C:(e + 1) * CH1C]
        in_maps.append(m)
    return in_maps


def _need_bias(inputs) -> bool:
    return any(
        float(np.abs(np.asarray(inputs[k])).max()) != 0.0
        for k in ("b1", "b2", "sw_b1", "sw_b2", "ch_b1")
    )


def run(inputs: dict, **run_kwargs):
    """Run on the 8 cores; returns (full_output, BassKernelResults)."""
    include_bias = _need_bias(inputs)
    nc = _get_nc(include_bias)
    in_maps = _pack_inputs(inputs, include_bias)
    res = run_bass_kernel_spmd(nc, in_maps, core_ids=list(range(NCORES)), **run_kwargs)
    out = np.zeros((B, C), dtype=np.float64)
    for e in range(NCORES):
        out += res.results[e]["outp"].astype(np.float64)
    out += np.asarray(inputs["ch_b2"], np.float64)
    return out.astype(np.float32), res


def kernel(**inputs) -> np.ndarray:
    out, _ = run(inputs)
    return out
